# revision 42
# baseline (speedup 1.0000x reference)
"""Causal multi-head attention (B=2, S=2048, D=2048, 32 heads x 64) for 8
Trainium2 NeuronCores.

Sharding: data parallel on batch (2 groups of 4 cores) x tensor parallel on
heads (4 groups of 8 heads each). Each core computes q/k/v projections for
its head group, RoPE, causal attention with sigmoid-gated values, and a
partial o-projection; the host sums the 4 partials per batch (the
"all-reduce" of the o-projection) and adds the output bias.

Per-core kernel design:
- All matmuls in float32r: full PE rate (1 cyc/row at N>=256) at ~1e-4
  relative precision, vs 4x slower fp32 and 16x less precise bf16.
- Layouts chosen so no on-device transposes are ever needed: x is fed
  pre-transposed [D, S]; q/k are produced head-major transposed
  ([head*64+d, s]); v is produced row-major [s, head*65+d] with a ones
  column per head so the softmax denominator rides the attention matmul
  for free (M=65); scores are computed transposed ([keys, queries]) so
  the A@V contraction needs no transpose either.
- RoPE is fused into the projection PSUM->SBUF move on the vector engine
  (partition-shifted multiplies with a sign-folded sin table); the 1/8
  attention scale is folded into the K weights on the host.
- Causal masking: 512-query blocks process only their <= keys; the 4
  diagonal key tiles get a -1e30 mask add in PSUM before the exp.
- Softmax denominators: reciprocal_approx_fast (from SBUF; ~18 bits) +
  gpsimd partition_broadcast, normalizing straight out of PSUM.
- DMAs use host-preblocked contiguous layouts chunked ~1MB across HWDGE
  queues; weights/x stream through double/triple-buffered pools.

Measured on 8 axon-tunneled trn2 cores: ~608-615 us mean / ~630 us max
core HW exec,
output absmax relative error ~2.3e-4 vs the fp32 reference.
"""

import os

import numpy as np

import concourse.bacc as bacc
import concourse.tile as tile
from concourse import mybir
from concourse.bass_utils import run_bass_kernel_spmd

B, S, D = 2, 2048, 2048
H_PER_CORE = 8          # heads per core
DH = 64                 # head dim
CW = 512                # per-core projection width = H_PER_CORE * DH
N_CORES = 8
KT = D // 128           # k-subtiles for the D-contraction

f32 = mybir.dt.float32
f32r = mybir.dt.float32r
Act = mybir.ActivationFunctionType

TRACE = bool(int(os.environ.get("KERNEL_TRACE", "0")))
LAST_EXEC_NS = None
LAST_MEAN_NS = None


def _build(WITH_BIAS=True):
    nc = bacc.Bacc("TRN2", target_bir_lowering=False, debug=False)

    x4 = nc.dram_tensor("x4", [4, 128, KT, 512], f32, kind="ExternalInput")
    wq4 = nc.dram_tensor("wq4", [4, 128, KT, 128], f32, kind="ExternalInput")
    wk4 = nc.dram_tensor("wk4", [4, 128, KT, 128], f32, kind="ExternalInput")
    wv4 = nc.dram_tensor("wv4", [128, KT, CW], f32, kind="ExternalInput")
    wo4 = nc.dram_tensor("wo4", [4, 128, 4, 512], f32, kind="ExternalInput")
    bq = nc.dram_tensor("bq", [1, CW], f32, kind="ExternalInput")
    bk = nc.dram_tensor("bk", [1, CW], f32, kind="ExternalInput")
    bv = nc.dram_tensor("bv", [1, CW], f32, kind="ExternalInput")
    rope4 = nc.dram_tensor("rope4", [4, 128, 512], f32, kind="ExternalInput")
    masks = nc.dram_tensor("masks", [128, 4, 512], f32, kind="ExternalInput")
    vinit = nc.dram_tensor("vinit", [128, 16 * 520], f32, kind="ExternalInput")
    part = nc.dram_tensor("part", [S, D], f32, kind="ExternalOutput")

    with tile.TileContext(nc) as tc:
        with (
            tc.tile_pool(name="p0", bufs=1) as p0,
            tc.tile_pool(name="pqk", bufs=1) as pqk,
        ):
            # persistent state (flat so wide RoPE ops can span M-tile pairs)
            qt_all = pqk.tile([128, 4, S], f32r, name="qt_all")
            kt_all = pqk.tile([128, 4, S], f32r, name="kt_all")
            qt = [qt_all[:, i, :] for i in range(4)]
            kt = [kt_all[:, i, :] for i in range(4)]
            # one flat tile: 16 x [128, 520] v-slabs, then row-consts
            # (SBUF tiles pad to 4KB/partition, so small tiles are wasteful)
            va_all = p0.tile([128, 16 * 520 + 4 * CW], f32r, name="va_all")
            va = [va_all[:, 520 * i:520 * (i + 1)] for i in range(16)]
            ones = va_all[0:1, 8320:8320 + CW]
            bvt = va_all[0:1, 8832:8832 + CW]
            bqrow = va_all[0:1, 9344:9344 + CW]
            bkrow = va_all[0:1, 9856:9856 + CW]

            if WITH_BIAS:
                nc.sync.dma_start(ones, vinit[0:1, 0:CW].bitcast(f32r))
                nc.sync.dma_start(bvt, bv[:].bitcast(f32r))
                nc.sync.dma_start(bqrow, bq[:].bitcast(f32r))
                nc.sync.dma_start(bkrow, bk[:].bitcast(f32r))

            # ---------------- Phase A1: Q/K projections ----------------
            with (
                tc.tile_pool(name="pa", bufs=2) as pa,
                tc.tile_pool(name="paw", bufs=3) as paw,
                tc.tile_pool(name="prc", bufs=2) as prc,
                tc.tile_pool(name="prt", bufs=2) as prt,
                tc.tile_pool(name="psa", bufs=4, space="PSUM") as psa,
            ):
                for qtr in range(4):
                    xtqA = pa.tile([128, KT // 2, 512], f32r, tag="xtA",
                                   name="xtqA")
                    xtqB = pa.tile([128, KT // 2, 512], f32r, tag="xtB",
                                   name="xtqB")
                    for kg in range(2):
                        nc.sync.dma_start(
                            xtqA[:, 4 * kg:4 * kg + 4, :],
                            x4[qtr, :, 4 * kg:4 * kg + 4, :].bitcast(f32r))
                        nc.sync.dma_start(
                            xtqB[:, 4 * kg:4 * kg + 4, :],
                            x4[qtr, :, 8 + 4 * kg:8 + 4 * kg + 4, :].bitcast(f32r))
                    xk = lambda k: (xtqA[:, k, :] if k < 8 else xtqB[:, k - 8, :])
                    tbl = prc.tile([128, 512], f32, tag="tbl", name="tbl")
                    nc.sync.dma_start(tbl[:], rope4[qtr])
                    cosc = tbl[0:64, :]
                    rsin = tbl[64:128, :]
                    # RoPE fused into the PSUM->SBUF move on the vector
                    # engine; M-tiles paired into 2-bank PSUM tiles so each
                    # DVE op runs at 1024 free elements.
                    for w3, dall, brow in (
                        (wq4, qt_all, bqrow),
                        (wk4, kt_all, bkrow),
                    ):
                        for mp in range(2):
                            ps = psa.tile([128, 2, 512], f32, tag="psa",
                                          name="ps_a")
                            for sub in range(2):
                                mt = 2 * mp + sub
                                wchA = paw.tile([128, KT // 2, 128], f32r,
                                                tag="wchA", name="wchA")
                                wchB = paw.tile([128, KT // 2, 128], f32r,
                                                tag="wchB", name="wchB")
                                nc.sync.dma_start(
                                    wchA[:], w3[mt, :, 0:8, :].bitcast(f32r))
                                nc.sync.dma_start(
                                    wchB[:], w3[mt, :, 8:16, :].bitcast(f32r))
                                for k in range(KT):
                                    wc = wchA[:, k, :] if k < 8 else wchB[:, k - 8, :]
                                    nc.tensor.matmul(
                                        ps[:, sub, :], wc,
                                        xk(k),
                                        start=(k == 0),
                                        stop=(k == KT - 1 and not WITH_BIAS),
                                    )
                                if WITH_BIAS:
                                    nc.tensor.matmul(
                                        ps[:, sub, :],
                                        brow[:, mt * 128:(mt + 1) * 128],
                                        ones, start=False, stop=True,
                                    )
                            cs = slice(qtr * 512, qtr * 512 + 512)
                            d = dall[:, 2 * mp:2 * mp + 2, cs]
                            c2 = cosc[:, None, :].to_broadcast((64, 2, 512))
                            nc.vector.tensor_mul(d[0:64], ps[0:64], c2)
                            nc.vector.tensor_mul(d[64:128], ps[64:128], c2)
                            # rotate-half * sin (sign folded into rsin)
                            tmp = prt.tile([128, 2, 512], f32r, tag="tmp",
                                           name="tmp")
                            r2a = rsin[0:32, None, :].to_broadcast((32, 2, 512))
                            r2b = rsin[32:64, None, :].to_broadcast((32, 2, 512))
                            for b0 in (0, 64):
                                nc.vector.tensor_mul(
                                    tmp[b0:b0 + 32], ps[b0 + 32:b0 + 64], r2a)
                                nc.vector.tensor_mul(
                                    tmp[b0 + 32:b0 + 64], ps[b0:b0 + 32], r2b)
                            nc.vector.tensor_add(d[:], d[:], tmp[:])

            with (
                tc.tile_pool(name="pav", bufs=1) as pav,
                tc.tile_pool(name="pa2", bufs=2) as pa2,
                tc.tile_pool(name="psv", bufs=3, space="PSUM") as psv,
            ):
                # va default 1.0 -> the per-head 65th column stays 1 (ones
                # column for the softmax denominator); data columns are
                # overwritten by the sigmoid copies below.
                for vg in range(4):
                    nc.sync.dma_start(
                        va_all[:, vg * 2080:(vg + 1) * 2080],
                        vinit[:, vg * 2080:(vg + 1) * 2080].bitcast(f32r))
                wvf = pav.tile([128, KT, CW], f32r, name="wvf")
                for kg in range(4):
                    nc.sync.dma_start(wvf[:, 4 * kg:4 * kg + 4, :],
                                      wv4[:, 4 * kg:4 * kg + 4, :].bitcast(f32r))
                for qtr in range(4):
                    xtqA = pa2.tile([128, KT // 2, 512], f32r, tag="xt2A",
                                    name="xtq2A")
                    xtqB = pa2.tile([128, KT // 2, 512], f32r, tag="xt2B",
                                    name="xtq2B")
                    for kg in range(2):
                        nc.sync.dma_start(
                            xtqA[:, 4 * kg:4 * kg + 4, :],
                            x4[qtr, :, 4 * kg:4 * kg + 4, :].bitcast(f32r))
                        nc.sync.dma_start(
                            xtqB[:, 4 * kg:4 * kg + 4, :],
                            x4[qtr, :, 8 + 4 * kg:8 + 4 * kg + 4, :].bitcast(f32r))
                    xk2 = lambda k: (xtqA[:, k, :] if k < 8 else xtqB[:, k - 8, :])
                    for st in range(4):
                        stg = qtr * 4 + st
                        ps = psv.tile([128, CW], f32, tag="psv", name="ps_v")
                        for k in range(KT):
                            nc.tensor.matmul(
                                ps[:], xk2(k)[:, st * 128:(st + 1) * 128],
                                wvf[:, k, :],
                                start=(k == 0),
                                stop=(k == KT - 1 and not WITH_BIAS),
                            )
                        if WITH_BIAS:
                            nc.tensor.matmul(
                                ps[:], ones[:, 0:128], bvt,
                                start=False, stop=True,
                            )
                        for h in range(H_PER_CORE):
                            nc.scalar.activation(
                                va[stg][:, 65 * h:65 * h + 64],
                                ps[:, 64 * h:64 * h + 64],
                                Act.Sigmoid,
                            )

            # ---------------- Phase B: causal attention ----------------
            with (
                tc.tile_pool(name="py", bufs=1) as py,
                tc.tile_pool(name="pc", bufs=2) as pc,
            ):
                ytr = [py.tile([128, S], f32r, name=f"ytr{i}") for i in range(4)]
                with (
                    tc.tile_pool(name="pb", bufs=1) as pb,
                    tc.tile_pool(name="pba", bufs=6) as pba,
                    tc.tile_pool(name="pbs", bufs=2) as pbs,
                    tc.tile_pool(name="pss", bufs=4, space="PSUM") as pss,
                    tc.tile_pool(name="psy", bufs=2, space="PSUM") as psy,
                    tc.tile_pool(name="pso", bufs=2, space="PSUM") as pso,
                ):
                    maskt = pb.tile([128, 4, 512], f32, name="maskt")
                    nc.sync.dma_start(maskt[:], masks[:])
                    kscr = [pb.tile([128, S], f32r, name=f"kscr{i}")
                            for i in range(2)]
                    # zero the pad halves (x0.0 of initialized data emits
                    # f32r-rounded zeros, which gpsimd memset cannot)
                    nc.vector.tensor_scalar_mul(kscr[0][64:128, :], qt[0][64:128, :], 0.0)
                    nc.vector.tensor_scalar_mul(kscr[1][0:64, :], qt[0][0:64, :], 0.0)

                    for qb in range(4):
                        nkt = 4 * qb + 4
                        for pi in range(4):
                            for hh in range(2):
                                lo, hi = hh * 64, (hh + 1) * 64
                                w = 512 * (qb + 1)
                                nc.vector.tensor_copy(
                                    kscr[hh][lo:hi, 0:w], kt[pi][lo:hi, 0:w]
                                )
                            for hh in range(2):
                                h = 2 * pi + hh
                                ks = kscr[hh]
                                lo, hi = hh * 64, (hh + 1) * 64
                                yps = psy.tile([65, 512], f32, tag="yps",
                                               name="ps_y")
                                for k_i in range(nkt):
                                    ps = pss.tile([128, 512], f32, tag="pss",
                                                  name="ps_s")
                                    nc.tensor.matmul(
                                        ps[:],
                                        ks[:, k_i * 128:(k_i + 1) * 128],
                                        qt[pi][:, qb * 512:(qb + 1) * 512],
                                        start=True, stop=True,
                                    )
                                    dt_i = k_i - 4 * qb
                                    if dt_i >= 0:
                                        nc.vector.tensor_add(
                                            ps[:], ps[:], maskt[:, dt_i, :]
                                        )
                                    at = pba.tile([128, 512], f32r, tag="at",
                                                  name="at")
                                    nc.scalar.activation(at[:], ps[:], Act.Exp)
                                    nc.tensor.matmul(
                                        yps[:],
                                        va[k_i][:, 65 * h:65 * h + 65],
                                        at[:],
                                        start=(k_i == 0), stop=(k_i == nkt - 1),
                                    )
                                den = pbs.tile([1, 512], f32, tag="den", name="den")
                                nc.vector.tensor_copy(den[:], yps[64:65, :])
                                rc = pbs.tile([1, 512], f32, tag="rc", name="rc")
                                nc.vector.reciprocal_approx_fast(rc[:], den[:])
                                s64 = pbs.tile([64, 512], f32, tag="s64",
                                               name="s64")
                                nc.gpsimd.partition_broadcast(s64[:], rc[:])
                                nc.vector.tensor_mul(
                                    ytr[pi][lo:hi, qb * 512:(qb + 1) * 512],
                                    yps[0:64, :], s64[:],
                                )
                        # o-projection rows for this query block are complete
                        for nt in range(4):
                            woc = pc.tile([128, 4, 512], f32r, tag="woc",
                                          name="woc")
                            for kc in range(4):
                                nc.sync.dma_start(
                                    woc[:, kc, :], wo4[nt, :, kc, :].bitcast(f32r))
                            for sl in range(4):
                                st = 4 * qb + sl
                                ps = pso.tile([128, 512], f32, tag="pso",
                                              name="ps_o")
                                for kc in range(4):
                                    nc.tensor.matmul(
                                        ps[:],
                                        ytr[kc][:, st * 128:(st + 1) * 128],
                                        woc[:, kc, :],
                                        start=(kc == 0), stop=(kc == 3),
                                    )
                                ostg = pc.tile([128, 512], f32, tag="ostg",
                                               name="ostg")
                                nc.scalar.copy(ostg[:], ps[:])
                                nc.sync.dma_start(
                                    part[st * 128:(st + 1) * 128,
                                         nt * 512:(nt + 1) * 512],
                                    ostg[:],
                                )

    nc.compile()
    return nc


def _rope_tables():
    half = DH // 2
    inv_freq = 1.0 / (10000.0 ** (np.arange(0, half, dtype=np.float32) / half))
    t = np.arange(S, dtype=np.float32)
    freqs = np.einsum("i,j->ij", t, inv_freq)            # [S, 32]
    emb = np.concatenate([freqs, freqs], axis=-1)        # [S, 64]
    cos = np.cos(emb).T.astype(np.float32)                        # [64, S]
    sin = np.sin(emb).T.astype(np.float32)
    rsin = np.concatenate([-sin[:32], sin[32:]], axis=0)
    return np.ascontiguousarray(np.concatenate([cos, rsin], axis=0))  # [128, S]


def _masks():
    j = np.arange(128)[:, None, None]
    dt = np.arange(4)[None, :, None]
    i = np.arange(512)[None, None, :]
    keep = (128 * dt + j) <= i
    return np.where(keep, 0.0, -1e30).astype(np.float32)  # [128, 4, 512]


def kernel(**inputs):
    global LAST_EXEC_NS
    x = np.asarray(inputs["x"], dtype=np.float32)
    Wq = np.asarray(inputs["Wq"], dtype=np.float32)
    Wk = np.asarray(inputs["Wk"], dtype=np.float32)
    Wv = np.asarray(inputs["Wv"], dtype=np.float32)
    Wo = np.asarray(inputs["Wo"], dtype=np.float32)
    bq = np.asarray(inputs["bq"], dtype=np.float32)
    bk = np.asarray(inputs["bk"], dtype=np.float32)
    bv = np.asarray(inputs["bv"], dtype=np.float32)
    bo = np.asarray(inputs["bo"], dtype=np.float32)

    ropeT = _rope_tables()
    masks = _masks()

    with_bias = any(float(np.abs(b).max()) > 0 for b in (bq, bk, bv))
    nc = _build(WITH_BIAS=with_bias)
    rope4 = np.ascontiguousarray(ropeT.reshape(128, 4, 512).transpose(1, 0, 2))
    vinit = np.ones((128, 16 * 520), dtype=np.float32)
    in_maps = []
    for c in range(N_CORES):
        b, g = c // 4, c % 4
        sl = slice(CW * g, CW * (g + 1))
        xT = x[b].T                                    # [D, S]
        x4 = np.ascontiguousarray(
            xT.reshape(KT, 128, 4, 512).transpose(2, 1, 0, 3))
        wq4 = np.ascontiguousarray(
            Wq[sl].T.reshape(KT, 128, 4, 128).transpose(2, 1, 0, 3))
        wk4 = np.ascontiguousarray(
            (Wk[sl].T * 0.125).reshape(KT, 128, 4, 128).transpose(2, 1, 0, 3))
        wv4 = np.ascontiguousarray(
            Wv[sl].T.reshape(KT, 128, CW).transpose(1, 0, 2))
        wo4 = np.ascontiguousarray(
            Wo[:, sl].T.reshape(4, 128, 4, 512).transpose(2, 1, 0, 3))
        in_maps.append({
            "x4": x4,
            "wq4": wq4,
            "wk4": wk4,
            "wv4": wv4,
            "wo4": wo4,
            "bq": np.ascontiguousarray(bq[sl].reshape(1, CW)),
            "bk": np.ascontiguousarray((bk[sl] * 0.125).reshape(1, CW)),
            "bv": np.ascontiguousarray(bv[sl].reshape(1, CW)),
            "rope4": rope4,
            "vinit": vinit,
            "masks": masks,
        })

    kwargs = {}
    if TRACE:
        kwargs = dict(trace=True, trace_cores=list(range(N_CORES)),
                      stitch_traces=False)
    global LAST_MEAN_NS
    r = run_bass_kernel_spmd(nc, in_maps, list(range(N_CORES)), **kwargs)
    LAST_EXEC_NS = r.exec_time_ns
    LAST_MEAN_NS = r.mean_exec_time_ns

    out = np.empty((B, S, D), dtype=np.float32)
    for b in range(B):
        acc = r.results[4 * b]["part"].astype(np.float32).copy()
        for g in range(1, 4):
            acc += r.results[4 * b + g]["part"]
        out[b] = acc + bo
    return out


# revision 43
# speedup vs baseline: 1.1081x; 1.1081x over previous
"""Causal multi-head attention (B=2, S=2048, D=2048, 32 heads x 64) for 8
Trainium2 NeuronCores.

Sharding: data parallel on batch (2 groups of 4 cores) x tensor parallel on
heads (4 groups of 8 heads each). Each core computes q/k/v projections for
its head group, RoPE, causal attention with sigmoid-gated values, and a
partial o-projection; the host sums the 4 partials per batch (the
"all-reduce" of the o-projection) and adds the output bias.

Per-core kernel design:
- All matmuls in float32r: full PE rate (1 cyc/row at N>=256) at ~1e-4
  relative precision, vs 4x slower fp32 and 16x less precise bf16.
- Layouts chosen so no on-device transposes are ever needed: x is fed
  pre-transposed [D, S]; q/k are produced head-major transposed
  ([head*64+d, s]); v is produced row-major [s, head*65+d] with a ones
  column per head so the softmax denominator rides the attention matmul
  for free (M=65); scores are computed transposed ([keys, queries]) so
  the A@V contraction needs no transpose either.
- RoPE is fused into the projection PSUM->SBUF move on the vector engine
  (partition-shifted multiplies with a sign-folded sin table); the 1/8
  attention scale is folded into the K weights on the host.
- Causal masking: 512-query blocks process only their <= keys; the 4
  diagonal key tiles get a -1e30 mask add in PSUM before the exp.
- Softmax denominators: reciprocal_approx_fast (from SBUF; ~18 bits) +
  gpsimd partition_broadcast, normalizing straight out of PSUM.
- DMAs use host-preblocked contiguous layouts chunked ~1MB across HWDGE
  queues; weights/x stream through double/triple-buffered pools.

Measured on 8 axon-tunneled trn2 cores: ~608-615 us mean / ~630 us max
core HW exec,
output absmax relative error ~2.3e-4 vs the fp32 reference.
"""

import os

import numpy as np

import concourse.bacc as bacc
import concourse.tile as tile
from concourse import mybir
from concourse.bass_utils import run_bass_kernel_spmd

B, S, D = 2, 2048, 2048
H_PER_CORE = 8          # heads per core
DH = 64                 # head dim
CW = 512                # per-core projection width = H_PER_CORE * DH
N_CORES = 8
KT = D // 128           # k-subtiles for the D-contraction

f32 = mybir.dt.float32
f32r = mybir.dt.float32r
Act = mybir.ActivationFunctionType

TRACE = bool(int(os.environ.get("KERNEL_TRACE", "0")))
LAST_EXEC_NS = None
LAST_MEAN_NS = None


def _build(WITH_BIAS=True):
    nc = bacc.Bacc("TRN2", target_bir_lowering=False, debug=False)

    x4 = nc.dram_tensor("x4", [4, 128, KT, 512], f32, kind="ExternalInput")
    wq4 = nc.dram_tensor("wq4", [4, 128, KT, 128], f32, kind="ExternalInput")
    wk4 = nc.dram_tensor("wk4", [4, 128, KT, 128], f32, kind="ExternalInput")
    wv4 = nc.dram_tensor("wv4", [128, KT, CW], f32, kind="ExternalInput")
    wo4 = nc.dram_tensor("wo4", [4, 128, 4, 512], f32, kind="ExternalInput")
    bq = nc.dram_tensor("bq", [1, CW], f32, kind="ExternalInput")
    bk = nc.dram_tensor("bk", [1, CW], f32, kind="ExternalInput")
    bv = nc.dram_tensor("bv", [1, CW], f32, kind="ExternalInput")
    rope4 = nc.dram_tensor("rope4", [4, 128, 512], f32, kind="ExternalInput")
    masks = nc.dram_tensor("masks", [128, 4, 512], f32, kind="ExternalInput")
    vinit = nc.dram_tensor("vinit", [128, 16 * 520], f32, kind="ExternalInput")
    part = nc.dram_tensor("part", [S, D], f32, kind="ExternalOutput")

    with tile.TileContext(nc) as tc:
        with (
            tc.tile_pool(name="p0", bufs=1) as p0,
            tc.tile_pool(name="pqk", bufs=1) as pqk,
        ):
            # persistent state (flat so wide RoPE ops can span M-tile pairs)
            qt_all = pqk.tile([128, 4, S], f32r, name="qt_all")
            kt_all = pqk.tile([128, 4, S], f32r, name="kt_all")
            qt = [qt_all[:, i, :] for i in range(4)]
            kt = [kt_all[:, i, :] for i in range(4)]
            # one flat tile: 16 x [128, 520] v-slabs, then row-consts
            # (SBUF tiles pad to 4KB/partition, so small tiles are wasteful)
            va_all = p0.tile([128, 16 * 520 + 4 * CW], f32r, name="va_all")
            va = [va_all[:, 520 * i:520 * (i + 1)] for i in range(16)]
            ones = va_all[0:1, 8320:8320 + CW]
            bvt = va_all[0:1, 8832:8832 + CW]
            bqrow = va_all[0:1, 9344:9344 + CW]
            bkrow = va_all[0:1, 9856:9856 + CW]

            if WITH_BIAS:
                nc.sync.dma_start(ones, vinit[0:1, 0:CW].bitcast(f32r))
                nc.sync.dma_start(bvt, bv[:].bitcast(f32r))
                nc.sync.dma_start(bqrow, bq[:].bitcast(f32r))
                nc.sync.dma_start(bkrow, bk[:].bitcast(f32r))

            # ---------------- Phase A1: Q/K projections ----------------
            with (
                tc.tile_pool(name="pa", bufs=2) as pa,
                tc.tile_pool(name="paw", bufs=3) as paw,
                tc.tile_pool(name="prc", bufs=2) as prc,
                tc.tile_pool(name="prt", bufs=2) as prt,
                tc.tile_pool(name="psa", bufs=4, space="PSUM") as psa,
            ):
                for qtr in range(4):
                    xtqA = pa.tile([128, KT // 2, 512], f32r, tag="xtA",
                                   name="xtqA")
                    xtqB = pa.tile([128, KT // 2, 512], f32r, tag="xtB",
                                   name="xtqB")
                    for kg in range(2):
                        nc.sync.dma_start(
                            xtqA[:, 4 * kg:4 * kg + 4, :],
                            x4[qtr, :, 4 * kg:4 * kg + 4, :].bitcast(f32r))
                        nc.sync.dma_start(
                            xtqB[:, 4 * kg:4 * kg + 4, :],
                            x4[qtr, :, 8 + 4 * kg:8 + 4 * kg + 4, :].bitcast(f32r))
                    xk = lambda k: (xtqA[:, k, :] if k < 8 else xtqB[:, k - 8, :])
                    tbl = prc.tile([128, 512], f32, tag="tbl", name="tbl")
                    nc.sync.dma_start(tbl[:], rope4[qtr])
                    cosc = tbl[0:64, :]
                    rsin = tbl[64:128, :]
                    # RoPE fused into the PSUM->SBUF move on the vector
                    # engine; M-tiles paired into 2-bank PSUM tiles so each
                    # DVE op runs at 1024 free elements.
                    for w3, dall, brow in (
                        (wq4, qt_all, bqrow),
                        (wk4, kt_all, bkrow),
                    ):
                        for mp in range(2):
                            ps = psa.tile([128, 2, 512], f32, tag="psa",
                                          name="ps_a")
                            for sub in range(2):
                                mt = 2 * mp + sub
                                wchA = paw.tile([128, KT // 2, 128], f32r,
                                                tag="wchA", name="wchA")
                                wchB = paw.tile([128, KT // 2, 128], f32r,
                                                tag="wchB", name="wchB")
                                nc.sync.dma_start(
                                    wchA[:], w3[mt, :, 0:8, :].bitcast(f32r))
                                nc.sync.dma_start(
                                    wchB[:], w3[mt, :, 8:16, :].bitcast(f32r))
                                for k in range(KT):
                                    wc = wchA[:, k, :] if k < 8 else wchB[:, k - 8, :]
                                    nc.tensor.matmul(
                                        ps[:, sub, :], wc,
                                        xk(k),
                                        start=(k == 0),
                                        stop=(k == KT - 1 and not WITH_BIAS),
                                    )
                                if WITH_BIAS:
                                    nc.tensor.matmul(
                                        ps[:, sub, :],
                                        brow[:, mt * 128:(mt + 1) * 128],
                                        ones, start=False, stop=True,
                                    )
                            cs = slice(qtr * 512, qtr * 512 + 512)
                            d = dall[:, 2 * mp:2 * mp + 2, cs]
                            c2 = cosc[:, None, :].to_broadcast((64, 2, 512))
                            nc.vector.tensor_mul(d[0:64], ps[0:64], c2)
                            nc.vector.tensor_mul(d[64:128], ps[64:128], c2)
                            # rotate-half * sin (sign folded into rsin)
                            tmp = prt.tile([128, 2, 512], f32r, tag="tmp",
                                           name="tmp")
                            r2a = rsin[0:32, None, :].to_broadcast((32, 2, 512))
                            r2b = rsin[32:64, None, :].to_broadcast((32, 2, 512))
                            for b0 in (0, 64):
                                nc.vector.tensor_mul(
                                    tmp[b0:b0 + 32], ps[b0 + 32:b0 + 64], r2a)
                                nc.vector.tensor_mul(
                                    tmp[b0 + 32:b0 + 64], ps[b0:b0 + 32], r2b)
                            nc.vector.tensor_add(d[:], d[:], tmp[:])

            with (
                tc.tile_pool(name="pav", bufs=1) as pav,
                tc.tile_pool(name="pa2", bufs=2) as pa2,
                tc.tile_pool(name="psv", bufs=3, space="PSUM") as psv,
            ):
                # va default 1.0 -> the per-head 65th column stays 1 (ones
                # column for the softmax denominator); data columns are
                # overwritten by the sigmoid copies below.
                for vg in range(4):
                    nc.sync.dma_start(
                        va_all[:, vg * 2080:(vg + 1) * 2080],
                        vinit[:, vg * 2080:(vg + 1) * 2080].bitcast(f32r))
                wvf = pav.tile([128, KT, CW], f32r, name="wvf")
                for kg in range(4):
                    nc.sync.dma_start(wvf[:, 4 * kg:4 * kg + 4, :],
                                      wv4[:, 4 * kg:4 * kg + 4, :].bitcast(f32r))
                for qtr in range(4):
                    xtqA = pa2.tile([128, KT // 2, 512], f32r, tag="xt2A",
                                    name="xtq2A")
                    xtqB = pa2.tile([128, KT // 2, 512], f32r, tag="xt2B",
                                    name="xtq2B")
                    for kg in range(2):
                        nc.sync.dma_start(
                            xtqA[:, 4 * kg:4 * kg + 4, :],
                            x4[qtr, :, 4 * kg:4 * kg + 4, :].bitcast(f32r))
                        nc.sync.dma_start(
                            xtqB[:, 4 * kg:4 * kg + 4, :],
                            x4[qtr, :, 8 + 4 * kg:8 + 4 * kg + 4, :].bitcast(f32r))
                    xk2 = lambda k: (xtqA[:, k, :] if k < 8 else xtqB[:, k - 8, :])
                    for st in range(4):
                        stg = qtr * 4 + st
                        ps = psv.tile([128, CW], f32, tag="psv", name="ps_v")
                        for k in range(KT):
                            nc.tensor.matmul(
                                ps[:], xk2(k)[:, st * 128:(st + 1) * 128],
                                wvf[:, k, :],
                                start=(k == 0),
                                stop=(k == KT - 1 and not WITH_BIAS),
                            )
                        if WITH_BIAS:
                            nc.tensor.matmul(
                                ps[:], ones[:, 0:128], bvt,
                                start=False, stop=True,
                            )
                        for h in range(H_PER_CORE):
                            nc.scalar.activation(
                                va[stg][:, 65 * h:65 * h + 64],
                                ps[:, 64 * h:64 * h + 64],
                                Act.Sigmoid,
                            )

            # ---------------- Phase B: causal attention ----------------
            with (
                tc.tile_pool(name="py", bufs=1) as py,
                tc.tile_pool(name="pc", bufs=2) as pc,
            ):
                ytr = [py.tile([128, S], f32r, name=f"ytr{i}") for i in range(4)]
                with (
                    tc.tile_pool(name="pb", bufs=1) as pb,
                    tc.tile_pool(name="pba", bufs=6) as pba,
                    tc.tile_pool(name="pbs", bufs=2) as pbs,
                    tc.tile_pool(name="pss", bufs=4, space="PSUM") as pss,
                    tc.tile_pool(name="psy", bufs=2, space="PSUM") as psy,
                    tc.tile_pool(name="pso", bufs=2, space="PSUM") as pso,
                ):
                    maskt = pb.tile([128, 4, 512], f32, name="maskt")
                    nc.sync.dma_start(maskt[:], masks[:])
                    kscr = [pb.tile([128, S], f32r, name=f"kscr{i}")
                            for i in range(2)]
                    # zero the pad halves (x0.0 of initialized data emits
                    # f32r-rounded zeros, which gpsimd memset cannot)
                    nc.vector.tensor_scalar_mul(kscr[0][64:128, :], qt[0][64:128, :], 0.0)
                    nc.vector.tensor_scalar_mul(kscr[1][0:64, :], qt[0][0:64, :], 0.0)

                    for qb in range(4):
                        nkt = 4 * qb + 4
                        for pi in range(4):
                            for hh in range(2):
                                lo, hi = hh * 64, (hh + 1) * 64
                                nc.vector.tensor_copy(
                                    kscr[hh][lo:hi, :], kt[pi][lo:hi, :]
                                )
                            for hh in range(2):
                                h = 2 * pi + hh
                                ks = kscr[hh]
                                lo, hi = hh * 64, (hh + 1) * 64
                                yps = psy.tile([65, 512], f32, tag="yps",
                                               name="ps_y")
                                for k_i in range(nkt):
                                    ps = pss.tile([128, 512], f32, tag="pss",
                                                  name="ps_s")
                                    nc.tensor.matmul(
                                        ps[:],
                                        ks[:, k_i * 128:(k_i + 1) * 128],
                                        qt[pi][:, qb * 512:(qb + 1) * 512],
                                        start=True, stop=True,
                                    )
                                    dt_i = k_i - 4 * qb
                                    if dt_i >= 0:
                                        nc.vector.tensor_add(
                                            ps[:], ps[:], maskt[:, dt_i, :]
                                        )
                                    at = pba.tile([128, 512], f32r, tag="at",
                                                  name="at")
                                    nc.scalar.activation(at[:], ps[:], Act.Exp)
                                    nc.tensor.matmul(
                                        yps[:],
                                        va[k_i][:, 65 * h:65 * h + 65],
                                        at[:],
                                        start=(k_i == 0), stop=(k_i == nkt - 1),
                                    )
                                den = pbs.tile([1, 512], f32, tag="den", name="den")
                                nc.vector.tensor_copy(den[:], yps[64:65, :])
                                rc = pbs.tile([1, 512], f32, tag="rc", name="rc")
                                nc.vector.reciprocal_approx_fast(rc[:], den[:])
                                s64 = pbs.tile([64, 512], f32, tag="s64",
                                               name="s64")
                                nc.gpsimd.partition_broadcast(s64[:], rc[:])
                                nc.vector.tensor_mul(
                                    ytr[pi][lo:hi, qb * 512:(qb + 1) * 512],
                                    yps[0:64, :], s64[:],
                                )
                        # o-projection rows for this query block are complete
                        for nt in range(4):
                            woc = pc.tile([128, 4, 512], f32r, tag="woc",
                                          name="woc")
                            for kc in range(4):
                                nc.sync.dma_start(
                                    woc[:, kc, :], wo4[nt, :, kc, :].bitcast(f32r))
                            for sl in range(4):
                                st = 4 * qb + sl
                                ps = pso.tile([128, 512], f32, tag="pso",
                                              name="ps_o")
                                for kc in range(4):
                                    nc.tensor.matmul(
                                        ps[:],
                                        ytr[kc][:, st * 128:(st + 1) * 128],
                                        woc[:, kc, :],
                                        start=(kc == 0), stop=(kc == 3),
                                    )
                                ostg = pc.tile([128, 512], f32, tag="ostg",
                                               name="ostg")
                                nc.scalar.copy(ostg[:], ps[:])
                                nc.sync.dma_start(
                                    part[st * 128:(st + 1) * 128,
                                         nt * 512:(nt + 1) * 512],
                                    ostg[:],
                                )

    nc.compile()
    return nc


def _rope_tables():
    half = DH // 2
    inv_freq = 1.0 / (10000.0 ** (np.arange(0, half, dtype=np.float32) / half))
    t = np.arange(S, dtype=np.float32)
    freqs = np.einsum("i,j->ij", t, inv_freq)            # [S, 32]
    emb = np.concatenate([freqs, freqs], axis=-1)        # [S, 64]
    cos = np.cos(emb).T.astype(np.float32)                        # [64, S]
    sin = np.sin(emb).T.astype(np.float32)
    rsin = np.concatenate([-sin[:32], sin[32:]], axis=0)
    return np.ascontiguousarray(np.concatenate([cos, rsin], axis=0))  # [128, S]


def _masks():
    j = np.arange(128)[:, None, None]
    dt = np.arange(4)[None, :, None]
    i = np.arange(512)[None, None, :]
    keep = (128 * dt + j) <= i
    return np.where(keep, 0.0, -1e30).astype(np.float32)  # [128, 4, 512]


def kernel(**inputs):
    global LAST_EXEC_NS
    x = np.asarray(inputs["x"], dtype=np.float32)
    Wq = np.asarray(inputs["Wq"], dtype=np.float32)
    Wk = np.asarray(inputs["Wk"], dtype=np.float32)
    Wv = np.asarray(inputs["Wv"], dtype=np.float32)
    Wo = np.asarray(inputs["Wo"], dtype=np.float32)
    bq = np.asarray(inputs["bq"], dtype=np.float32)
    bk = np.asarray(inputs["bk"], dtype=np.float32)
    bv = np.asarray(inputs["bv"], dtype=np.float32)
    bo = np.asarray(inputs["bo"], dtype=np.float32)

    ropeT = _rope_tables()
    masks = _masks()

    with_bias = any(float(np.abs(b).max()) > 0 for b in (bq, bk, bv))
    nc = _build(WITH_BIAS=with_bias)
    rope4 = np.ascontiguousarray(ropeT.reshape(128, 4, 512).transpose(1, 0, 2))
    vinit = np.ones((128, 16 * 520), dtype=np.float32)
    in_maps = []
    for c in range(N_CORES):
        b, g = c // 4, c % 4
        sl = slice(CW * g, CW * (g + 1))
        xT = x[b].T                                    # [D, S]
        x4 = np.ascontiguousarray(
            xT.reshape(KT, 128, 4, 512).transpose(2, 1, 0, 3))
        wq4 = np.ascontiguousarray(
            Wq[sl].T.reshape(KT, 128, 4, 128).transpose(2, 1, 0, 3))
        wk4 = np.ascontiguousarray(
            (Wk[sl].T * 0.125).reshape(KT, 128, 4, 128).transpose(2, 1, 0, 3))
        wv4 = np.ascontiguousarray(
            Wv[sl].T.reshape(KT, 128, CW).transpose(1, 0, 2))
        wo4 = np.ascontiguousarray(
            Wo[:, sl].T.reshape(4, 128, 4, 512).transpose(2, 1, 0, 3))
        in_maps.append({
            "x4": x4,
            "wq4": wq4,
            "wk4": wk4,
            "wv4": wv4,
            "wo4": wo4,
            "bq": np.ascontiguousarray(bq[sl].reshape(1, CW)),
            "bk": np.ascontiguousarray((bk[sl] * 0.125).reshape(1, CW)),
            "bv": np.ascontiguousarray(bv[sl].reshape(1, CW)),
            "rope4": rope4,
            "vinit": vinit,
            "masks": masks,
        })

    kwargs = {}
    if TRACE:
        kwargs = dict(trace=True, trace_cores=list(range(N_CORES)),
                      stitch_traces=False)
    global LAST_MEAN_NS
    r = run_bass_kernel_spmd(nc, in_maps, list(range(N_CORES)), **kwargs)
    LAST_EXEC_NS = r.exec_time_ns
    LAST_MEAN_NS = r.mean_exec_time_ns

    out = np.empty((B, S, D), dtype=np.float32)
    for b in range(B):
        acc = r.results[4 * b]["part"].astype(np.float32).copy()
        for g in range(1, 4):
            acc += r.results[4 * b + g]["part"]
        out[b] = acc + bo
    return out


# revision 45
# speedup vs baseline: 1.1359x; 1.0251x over previous
"""Causal multi-head attention (B=2, S=2048, D=2048, 32 heads x 64) for 8
Trainium2 NeuronCores.

Sharding: data parallel on batch (2 groups of 4 cores) x tensor parallel on
heads (4 groups of 8 heads each). Each core computes q/k/v projections for
its head group, RoPE, causal attention with sigmoid-gated values, and a
partial o-projection; the host sums the 4 partials per batch (the
"all-reduce" of the o-projection) and adds the output bias.

Per-core kernel design:
- All matmuls in float32r: full PE rate (1 cyc/row at N>=256) at ~1e-4
  relative precision, vs 4x slower fp32 and 16x less precise bf16.
- Layouts chosen so no on-device transposes are ever needed: x is fed
  pre-transposed [D, S]; q/k are produced head-major transposed
  ([head*64+d, s]); v is produced row-major [s, head*65+d] with a ones
  column per head so the softmax denominator rides the attention matmul
  for free (M=65); scores are computed transposed ([keys, queries]) so
  the A@V contraction needs no transpose either.
- RoPE is fused into the projection PSUM->SBUF move on the vector engine
  (partition-shifted multiplies with a sign-folded sin table); the 1/8
  attention scale is folded into the K weights on the host.
- Causal masking: 512-query blocks process only their <= keys; the 4
  diagonal key tiles get a -1e30 mask add in PSUM before the exp.
- Softmax denominators: reciprocal_approx_fast (from SBUF; ~18 bits) +
  gpsimd partition_broadcast, normalizing straight out of PSUM.
- DMAs use host-preblocked contiguous layouts chunked ~1MB across HWDGE
  queues; weights/x stream through double/triple-buffered pools.

Measured on 8 axon-tunneled trn2 cores: ~608-615 us mean / ~630 us max
core HW exec,
output absmax relative error ~2.3e-4 vs the fp32 reference.
"""

import os

import numpy as np

import concourse.bacc as bacc
import concourse.tile as tile
from concourse import mybir
from concourse.bass_utils import run_bass_kernel_spmd

B, S, D = 2, 2048, 2048
H_PER_CORE = 8          # heads per core
DH = 64                 # head dim
CW = 512                # per-core projection width = H_PER_CORE * DH
N_CORES = 8
KT = D // 128           # k-subtiles for the D-contraction

f32 = mybir.dt.float32
f32r = mybir.dt.float32r
Act = mybir.ActivationFunctionType

TRACE = bool(int(os.environ.get("KERNEL_TRACE", "0")))
LAST_EXEC_NS = None
LAST_MEAN_NS = None


def _build(WITH_BIAS=True):
    nc = bacc.Bacc("TRN2", target_bir_lowering=False, debug=False)

    x4 = nc.dram_tensor("x4", [4, 128, KT, 512], f32, kind="ExternalInput")
    wq4 = nc.dram_tensor("wq4", [4, 128, KT, 128], f32, kind="ExternalInput")
    wk4 = nc.dram_tensor("wk4", [4, 128, KT, 128], f32, kind="ExternalInput")
    wv4 = nc.dram_tensor("wv4", [128, KT, CW], f32, kind="ExternalInput")
    wo4 = nc.dram_tensor("wo4", [4, 128, 4, 512], f32, kind="ExternalInput")
    bq = nc.dram_tensor("bq", [1, CW], f32, kind="ExternalInput")
    bk = nc.dram_tensor("bk", [1, CW], f32, kind="ExternalInput")
    bv = nc.dram_tensor("bv", [1, CW], f32, kind="ExternalInput")
    rope4 = nc.dram_tensor("rope4", [4, 128, 512], f32, kind="ExternalInput")
    masks = nc.dram_tensor("masks", [128, 4, 512], f32, kind="ExternalInput")
    vinit = nc.dram_tensor("vinit", [128, 16 * 520], f32, kind="ExternalInput")
    part = nc.dram_tensor("part", [S, D], f32, kind="ExternalOutput")

    with tile.TileContext(nc) as tc:
        with (
            tc.tile_pool(name="p0", bufs=1) as p0,
            tc.tile_pool(name="pqk", bufs=1) as pqk,
        ):
            # persistent state (flat so wide RoPE ops can span M-tile pairs)
            qt_all = pqk.tile([128, 4, S], f32r, name="qt_all")
            kt_all = pqk.tile([128, 4, S], f32r, name="kt_all")
            qt = [qt_all[:, i, :] for i in range(4)]
            kt = [kt_all[:, i, :] for i in range(4)]
            # one flat tile: 16 x [128, 520] v-slabs, then row-consts
            # (SBUF tiles pad to 4KB/partition, so small tiles are wasteful)
            va_all = p0.tile([128, 16 * 520 + 4 * CW], f32r, name="va_all")
            va = [va_all[:, 520 * i:520 * (i + 1)] for i in range(16)]
            ones = va_all[0:1, 8320:8320 + CW]
            bvt = va_all[0:1, 8832:8832 + CW]
            bqrow = va_all[0:1, 9344:9344 + CW]
            bkrow = va_all[0:1, 9856:9856 + CW]

            if WITH_BIAS:
                nc.sync.dma_start(ones, vinit[0:1, 0:CW].bitcast(f32r))
                nc.sync.dma_start(bvt, bv[:].bitcast(f32r))
                nc.sync.dma_start(bqrow, bq[:].bitcast(f32r))
                nc.sync.dma_start(bkrow, bk[:].bitcast(f32r))

            # ---------------- Phase A1: Q/K projections ----------------
            with (
                tc.tile_pool(name="pa", bufs=2) as pa,
                tc.tile_pool(name="paw", bufs=3) as paw,
                tc.tile_pool(name="prc", bufs=1) as prc,
                tc.tile_pool(name="prt", bufs=2) as prt,
                tc.tile_pool(name="psa", bufs=4, space="PSUM") as psa,
            ):
                for half in range(2):
                    xts = []
                    for qloc in range(2):
                        qtr = 2 * half + qloc
                        xtqA = pa.tile([128, KT // 2, 512], f32r, tag="xtA",
                                       name="xtqA")
                        xtqB = pa.tile([128, KT // 2, 512], f32r, tag="xtB",
                                       name="xtqB")
                        for kg in range(2):
                            nc.sync.dma_start(
                                xtqA[:, 4 * kg:4 * kg + 4, :],
                                x4[qtr, :, 4 * kg:4 * kg + 4, :].bitcast(f32r))
                            nc.sync.dma_start(
                                xtqB[:, 4 * kg:4 * kg + 4, :],
                                x4[qtr, :, 8 + 4 * kg:8 + 4 * kg + 4, :].bitcast(f32r))
                        xts.append((xtqA, xtqB))
                    tblw = prc.tile([128, 2, 512], f32, tag="tbl", name="tblw")
                    for qloc in range(2):
                        nc.sync.dma_start(tblw[:, qloc, :], rope4[2 * half + qloc])
                    cosw = tblw[0:64]
                    r2a = tblw[64:96]
                    r2b = tblw[96:128]
                    # Each weight chunk loads once per half; its PSUM pairs the
                    # two quarters so RoPE runs at 1024 free elements per op.
                    for w3, dall, brow in (
                        (wq4, qt_all, bqrow),
                        (wk4, kt_all, bkrow),
                    ):
                        for mt in range(4):
                            wchA = paw.tile([128, KT // 2, 128], f32r,
                                            tag="wchA", name="wchA")
                            wchB = paw.tile([128, KT // 2, 128], f32r,
                                            tag="wchB", name="wchB")
                            nc.sync.dma_start(
                                wchA[:], w3[mt, :, 0:8, :].bitcast(f32r))
                            nc.sync.dma_start(
                                wchB[:], w3[mt, :, 8:16, :].bitcast(f32r))
                            ps = psa.tile([128, 2, 512], f32, tag="psa",
                                          name="ps_a")
                            for qloc in range(2):
                                xA, xB = xts[qloc]
                                for k in range(KT):
                                    wc = (wchA[:, k, :] if k < 8
                                          else wchB[:, k - 8, :])
                                    xv = (xA[:, k, :] if k < 8
                                          else xB[:, k - 8, :])
                                    nc.tensor.matmul(
                                        ps[:, qloc, :], wc, xv,
                                        start=(k == 0),
                                        stop=(k == KT - 1 and not WITH_BIAS),
                                    )
                                if WITH_BIAS:
                                    nc.tensor.matmul(
                                        ps[:, qloc, :],
                                        brow[:, mt * 128:(mt + 1) * 128],
                                        ones, start=False, stop=True,
                                    )
                            d3 = dall[:, mt, half * 1024:(half + 1) * 1024
                                      ].rearrange("p (a b) -> p a b", a=2)
                            nc.vector.tensor_mul(d3[0:64], ps[0:64], cosw)
                            nc.vector.tensor_mul(d3[64:128], ps[64:128], cosw)
                            # rotate-half * sin (sign folded into rsin)
                            tmp = prt.tile([128, 2, 512], f32r, tag="tmp",
                                           name="tmp")
                            for b0 in (0, 64):
                                nc.vector.tensor_mul(
                                    tmp[b0:b0 + 32], ps[b0 + 32:b0 + 64], r2a)
                                nc.vector.tensor_mul(
                                    tmp[b0 + 32:b0 + 64], ps[b0:b0 + 32], r2b)
                            nc.vector.tensor_add(d3[:], d3[:], tmp[:])

            with (
                tc.tile_pool(name="pav", bufs=1) as pav,
                tc.tile_pool(name="pa2", bufs=2) as pa2,
                tc.tile_pool(name="psv", bufs=3, space="PSUM") as psv,
            ):
                # va default 1.0 -> the per-head 65th column stays 1 (ones
                # column for the softmax denominator); data columns are
                # overwritten by the sigmoid copies below.
                for vg in range(4):
                    nc.sync.dma_start(
                        va_all[:, vg * 2080:(vg + 1) * 2080],
                        vinit[:, vg * 2080:(vg + 1) * 2080].bitcast(f32r))
                wvf = pav.tile([128, KT, CW], f32r, name="wvf")
                for kg in range(4):
                    nc.sync.dma_start(wvf[:, 4 * kg:4 * kg + 4, :],
                                      wv4[:, 4 * kg:4 * kg + 4, :].bitcast(f32r))
                for qtr in range(4):
                    xtqA = pa2.tile([128, KT // 2, 512], f32r, tag="xt2A",
                                    name="xtq2A")
                    xtqB = pa2.tile([128, KT // 2, 512], f32r, tag="xt2B",
                                    name="xtq2B")
                    for kg in range(2):
                        nc.sync.dma_start(
                            xtqA[:, 4 * kg:4 * kg + 4, :],
                            x4[qtr, :, 4 * kg:4 * kg + 4, :].bitcast(f32r))
                        nc.sync.dma_start(
                            xtqB[:, 4 * kg:4 * kg + 4, :],
                            x4[qtr, :, 8 + 4 * kg:8 + 4 * kg + 4, :].bitcast(f32r))
                    xk2 = lambda k: (xtqA[:, k, :] if k < 8 else xtqB[:, k - 8, :])
                    for st in range(4):
                        stg = qtr * 4 + st
                        ps = psv.tile([128, CW], f32, tag="psv", name="ps_v")
                        for k in range(KT):
                            nc.tensor.matmul(
                                ps[:], xk2(k)[:, st * 128:(st + 1) * 128],
                                wvf[:, k, :],
                                start=(k == 0),
                                stop=(k == KT - 1 and not WITH_BIAS),
                            )
                        if WITH_BIAS:
                            nc.tensor.matmul(
                                ps[:], ones[:, 0:128], bvt,
                                start=False, stop=True,
                            )
                        for h in range(H_PER_CORE):
                            nc.scalar.activation(
                                va[stg][:, 65 * h:65 * h + 64],
                                ps[:, 64 * h:64 * h + 64],
                                Act.Sigmoid,
                            )

            # ---------------- Phase B: causal attention ----------------
            with (
                tc.tile_pool(name="py", bufs=1) as py,
                tc.tile_pool(name="pc", bufs=2) as pc,
            ):
                ytr = [py.tile([128, S], f32r, name=f"ytr{i}") for i in range(4)]
                with (
                    tc.tile_pool(name="pb", bufs=1) as pb,
                    tc.tile_pool(name="pba", bufs=6) as pba,
                    tc.tile_pool(name="pbs", bufs=2) as pbs,
                    tc.tile_pool(name="pss", bufs=4, space="PSUM") as pss,
                    tc.tile_pool(name="psy", bufs=2, space="PSUM") as psy,
                    tc.tile_pool(name="pso", bufs=2, space="PSUM") as pso,
                ):
                    maskt = pb.tile([128, 4, 512], f32, name="maskt")
                    nc.sync.dma_start(maskt[:], masks[:])
                    kscr = [pb.tile([128, S], f32r, name=f"kscr{i}")
                            for i in range(2)]
                    # zero the pad halves (x0.0 of initialized data emits
                    # f32r-rounded zeros, which gpsimd memset cannot)
                    nc.vector.tensor_scalar_mul(kscr[0][64:128, :], qt[0][64:128, :], 0.0)
                    nc.vector.tensor_scalar_mul(kscr[1][0:64, :], qt[0][0:64, :], 0.0)

                    for qb in range(4):
                        nkt = 4 * qb + 4
                        for pi in range(4):
                            for hh in range(2):
                                lo, hi = hh * 64, (hh + 1) * 64
                                nc.vector.tensor_copy(
                                    kscr[hh][lo:hi, :], kt[pi][lo:hi, :]
                                )
                            for hh in range(2):
                                h = 2 * pi + hh
                                ks = kscr[hh]
                                lo, hi = hh * 64, (hh + 1) * 64
                                yps = psy.tile([65, 512], f32, tag="yps",
                                               name="ps_y")
                                for k_i in range(nkt):
                                    ps = pss.tile([128, 512], f32, tag="pss",
                                                  name="ps_s")
                                    nc.tensor.matmul(
                                        ps[:],
                                        ks[:, k_i * 128:(k_i + 1) * 128],
                                        qt[pi][:, qb * 512:(qb + 1) * 512],
                                        start=True, stop=True,
                                    )
                                    dt_i = k_i - 4 * qb
                                    if dt_i >= 0:
                                        nc.vector.tensor_add(
                                            ps[:], ps[:], maskt[:, dt_i, :]
                                        )
                                    at = pba.tile([128, 512], f32r, tag="at",
                                                  name="at")
                                    nc.scalar.activation(at[:], ps[:], Act.Exp)
                                    nc.tensor.matmul(
                                        yps[:],
                                        va[k_i][:, 65 * h:65 * h + 65],
                                        at[:],
                                        start=(k_i == 0), stop=(k_i == nkt - 1),
                                    )
                                den = pbs.tile([1, 512], f32, tag="den", name="den")
                                nc.vector.tensor_copy(den[:], yps[64:65, :])
                                rc = pbs.tile([1, 512], f32, tag="rc", name="rc")
                                nc.vector.reciprocal_approx_fast(rc[:], den[:])
                                s64 = pbs.tile([64, 512], f32, tag="s64",
                                               name="s64")
                                nc.gpsimd.partition_broadcast(s64[:], rc[:])
                                nc.vector.tensor_mul(
                                    ytr[pi][lo:hi, qb * 512:(qb + 1) * 512],
                                    yps[0:64, :], s64[:],
                                )
                        # o-projection rows for this query block are complete
                        for nt in range(4):
                            woc = pc.tile([128, 4, 512], f32r, tag="woc",
                                          name="woc")
                            for kc in range(4):
                                nc.sync.dma_start(
                                    woc[:, kc, :], wo4[nt, :, kc, :].bitcast(f32r))
                            for sl in range(4):
                                st = 4 * qb + sl
                                ps = pso.tile([128, 512], f32, tag="pso",
                                              name="ps_o")
                                for kc in range(4):
                                    nc.tensor.matmul(
                                        ps[:],
                                        ytr[kc][:, st * 128:(st + 1) * 128],
                                        woc[:, kc, :],
                                        start=(kc == 0), stop=(kc == 3),
                                    )
                                ostg = pc.tile([128, 512], f32, tag="ostg",
                                               name="ostg")
                                nc.scalar.copy(ostg[:], ps[:])
                                nc.sync.dma_start(
                                    part[st * 128:(st + 1) * 128,
                                         nt * 512:(nt + 1) * 512],
                                    ostg[:],
                                )

    nc.compile()
    return nc


def _rope_tables():
    half = DH // 2
    inv_freq = 1.0 / (10000.0 ** (np.arange(0, half, dtype=np.float32) / half))
    t = np.arange(S, dtype=np.float32)
    freqs = np.einsum("i,j->ij", t, inv_freq)            # [S, 32]
    emb = np.concatenate([freqs, freqs], axis=-1)        # [S, 64]
    cos = np.cos(emb).T.astype(np.float32)                        # [64, S]
    sin = np.sin(emb).T.astype(np.float32)
    rsin = np.concatenate([-sin[:32], sin[32:]], axis=0)
    return np.ascontiguousarray(np.concatenate([cos, rsin], axis=0))  # [128, S]


def _masks():
    j = np.arange(128)[:, None, None]
    dt = np.arange(4)[None, :, None]
    i = np.arange(512)[None, None, :]
    keep = (128 * dt + j) <= i
    return np.where(keep, 0.0, -1e30).astype(np.float32)  # [128, 4, 512]


def kernel(**inputs):
    global LAST_EXEC_NS
    x = np.asarray(inputs["x"], dtype=np.float32)
    Wq = np.asarray(inputs["Wq"], dtype=np.float32)
    Wk = np.asarray(inputs["Wk"], dtype=np.float32)
    Wv = np.asarray(inputs["Wv"], dtype=np.float32)
    Wo = np.asarray(inputs["Wo"], dtype=np.float32)
    bq = np.asarray(inputs["bq"], dtype=np.float32)
    bk = np.asarray(inputs["bk"], dtype=np.float32)
    bv = np.asarray(inputs["bv"], dtype=np.float32)
    bo = np.asarray(inputs["bo"], dtype=np.float32)

    ropeT = _rope_tables()
    masks = _masks()

    with_bias = any(float(np.abs(b).max()) > 0 for b in (bq, bk, bv))
    nc = _build(WITH_BIAS=with_bias)
    rope4 = np.ascontiguousarray(ropeT.reshape(128, 4, 512).transpose(1, 0, 2))
    vinit = np.ones((128, 16 * 520), dtype=np.float32)
    in_maps = []
    for c in range(N_CORES):
        b, g = c // 4, c % 4
        sl = slice(CW * g, CW * (g + 1))
        xT = x[b].T                                    # [D, S]
        x4 = np.ascontiguousarray(
            xT.reshape(KT, 128, 4, 512).transpose(2, 1, 0, 3))
        wq4 = np.ascontiguousarray(
            Wq[sl].T.reshape(KT, 128, 4, 128).transpose(2, 1, 0, 3))
        wk4 = np.ascontiguousarray(
            (Wk[sl].T * 0.125).reshape(KT, 128, 4, 128).transpose(2, 1, 0, 3))
        wv4 = np.ascontiguousarray(
            Wv[sl].T.reshape(KT, 128, CW).transpose(1, 0, 2))
        wo4 = np.ascontiguousarray(
            Wo[:, sl].T.reshape(4, 128, 4, 512).transpose(2, 1, 0, 3))
        in_maps.append({
            "x4": x4,
            "wq4": wq4,
            "wk4": wk4,
            "wv4": wv4,
            "wo4": wo4,
            "bq": np.ascontiguousarray(bq[sl].reshape(1, CW)),
            "bk": np.ascontiguousarray((bk[sl] * 0.125).reshape(1, CW)),
            "bv": np.ascontiguousarray(bv[sl].reshape(1, CW)),
            "rope4": rope4,
            "vinit": vinit,
            "masks": masks,
        })

    kwargs = {}
    if TRACE:
        kwargs = dict(trace=True, trace_cores=list(range(N_CORES)),
                      stitch_traces=False)
    global LAST_MEAN_NS
    r = run_bass_kernel_spmd(nc, in_maps, list(range(N_CORES)), **kwargs)
    LAST_EXEC_NS = r.exec_time_ns
    LAST_MEAN_NS = r.mean_exec_time_ns

    out = np.empty((B, S, D), dtype=np.float32)
    for b in range(B):
        acc = r.results[4 * b]["part"].astype(np.float32).copy()
        for g in range(1, 4):
            acc += r.results[4 * b + g]["part"]
        out[b] = acc + bo
    return out


# revision 47
# speedup vs baseline: 1.1681x; 1.0284x over previous
"""Causal multi-head attention (B=2, S=2048, D=2048, 32 heads x 64) for 8
Trainium2 NeuronCores.

Sharding: data parallel on batch (2 groups of 4 cores) x tensor parallel on
heads (4 groups of 8 heads each). Each core computes q/k/v projections for
its head group, RoPE, causal attention with sigmoid-gated values, and a
partial o-projection; the host sums the 4 partials per batch (the
"all-reduce" of the o-projection) and adds the output bias.

Per-core kernel design:
- All matmuls in float32r: full PE rate (1 cyc/row at N>=256) at ~1e-4
  relative precision, vs 4x slower fp32 and 16x less precise bf16.
- Layouts chosen so no on-device transposes are ever needed: x is fed
  pre-transposed [D, S]; q/k are produced head-major transposed
  ([head*64+d, s]); v is produced row-major [s, head*65+d] with a ones
  column per head so the softmax denominator rides the attention matmul
  for free (M=65); scores are computed transposed ([keys, queries]) so
  the A@V contraction needs no transpose either.
- RoPE is fused into the projection PSUM->SBUF move on the vector engine
  (partition-shifted multiplies with a sign-folded sin table); the 1/8
  attention scale is folded into the K weights on the host.
- Causal masking: 512-query blocks process only their <= keys; the 4
  diagonal key tiles get a -1e30 mask add in PSUM before the exp.
- Softmax denominators: reciprocal_approx_fast (from SBUF; ~18 bits) +
  gpsimd partition_broadcast, normalizing straight out of PSUM.
- DMAs use host-preblocked contiguous layouts chunked ~1MB across HWDGE
  queues; weights/x stream through double/triple-buffered pools. The Q/K
  projection keeps two x-quarters resident so each weight chunk loads
  once per half (not once per quarter), halving projection weight
  traffic; attention runs query-block-major so the o-projection for
  each 512-query block interleaves into the attention stream.

Measured on 8 axon-tunneled trn2 cores: ~605-610 us mean / ~621-623 us
max core HW exec, output absmax relative error ~2.3e-4 vs the fp32
reference.
"""

import os

import numpy as np

import concourse.bacc as bacc
import concourse.tile as tile
from concourse import mybir
from concourse.bass_utils import run_bass_kernel_spmd

B, S, D = 2, 2048, 2048
H_PER_CORE = 8          # heads per core
DH = 64                 # head dim
CW = 512                # per-core projection width = H_PER_CORE * DH
N_CORES = 8
KT = D // 128           # k-subtiles for the D-contraction

f32 = mybir.dt.float32
f32r = mybir.dt.float32r
Act = mybir.ActivationFunctionType

TRACE = bool(int(os.environ.get("KERNEL_TRACE", "0")))
LAST_EXEC_NS = None
LAST_MEAN_NS = None


def _build(WITH_BIAS=True):
    nc = bacc.Bacc("TRN2", target_bir_lowering=False, debug=False)

    x4 = nc.dram_tensor("x4", [4, 128, KT, 512], f32, kind="ExternalInput")
    wq4 = nc.dram_tensor("wq4", [4, 128, KT, 128], f32, kind="ExternalInput")
    wk4 = nc.dram_tensor("wk4", [4, 128, KT, 128], f32, kind="ExternalInput")
    wv4 = nc.dram_tensor("wv4", [128, KT, CW], f32, kind="ExternalInput")
    wo4 = nc.dram_tensor("wo4", [4, 128, 4, 512], f32, kind="ExternalInput")
    bq = nc.dram_tensor("bq", [1, CW], f32, kind="ExternalInput")
    bk = nc.dram_tensor("bk", [1, CW], f32, kind="ExternalInput")
    bv = nc.dram_tensor("bv", [1, CW], f32, kind="ExternalInput")
    rope4 = nc.dram_tensor("rope4", [4, 128, 512], f32, kind="ExternalInput")
    masks = nc.dram_tensor("masks", [128, 4, 512], f32, kind="ExternalInput")
    vinit = nc.dram_tensor("vinit", [128, 16 * 520], f32, kind="ExternalInput")
    part = nc.dram_tensor("part", [S, D], f32, kind="ExternalOutput")

    with tile.TileContext(nc) as tc:
        with (
            tc.tile_pool(name="p0", bufs=1) as p0,
            tc.tile_pool(name="pqk", bufs=1) as pqk,
        ):
            # persistent state (flat so wide RoPE ops can span M-tile pairs)
            qt_all = pqk.tile([128, 4, S], f32r, name="qt_all")
            kt_all = pqk.tile([128, 4, S], f32r, name="kt_all")
            qt = [qt_all[:, i, :] for i in range(4)]
            kt = [kt_all[:, i, :] for i in range(4)]
            # one flat tile: 16 x [128, 520] v-slabs, then row-consts
            # (SBUF tiles pad to 4KB/partition, so small tiles are wasteful)
            va_all = p0.tile([128, 16 * 520 + 4 * CW], f32r, name="va_all")
            va = [va_all[:, 520 * i:520 * (i + 1)] for i in range(16)]
            ones = va_all[0:1, 8320:8320 + CW]
            bvt = va_all[0:1, 8832:8832 + CW]
            bqrow = va_all[0:1, 9344:9344 + CW]
            bkrow = va_all[0:1, 9856:9856 + CW]

            if WITH_BIAS:
                nc.sync.dma_start(ones, vinit[0:1, 0:CW].bitcast(f32r))
                nc.sync.dma_start(bvt, bv[:].bitcast(f32r))
                nc.sync.dma_start(bqrow, bq[:].bitcast(f32r))
                nc.sync.dma_start(bkrow, bk[:].bitcast(f32r))

            # ---------------- Phase A1: Q/K projections ----------------
            with (
                tc.tile_pool(name="pa", bufs=2) as pa,
                tc.tile_pool(name="paw", bufs=3) as paw,
                tc.tile_pool(name="prc", bufs=1) as prc,
                tc.tile_pool(name="prt", bufs=2) as prt,
                tc.tile_pool(name="psa", bufs=4, space="PSUM") as psa,
            ):
                def _load_x(qtr):
                    xtqA = pa.tile([128, KT // 2, 512], f32r, tag="xtA",
                                   name="xtqA")
                    xtqB = pa.tile([128, KT // 2, 512], f32r, tag="xtB",
                                   name="xtqB")
                    for kg in range(2):
                        nc.sync.dma_start(
                            xtqA[:, 4 * kg:4 * kg + 4, :],
                            x4[qtr, :, 4 * kg:4 * kg + 4, :].bitcast(f32r))
                        nc.sync.dma_start(
                            xtqB[:, 4 * kg:4 * kg + 4, :],
                            x4[qtr, :, 8 + 4 * kg:8 + 4 * kg + 4, :].bitcast(f32r))
                    return (xtqA, xtqB)

                for half in range(2):
                    # q0's x first; q1's x is emitted after the first weight
                    # chunk below so the first matmuls aren't queued behind it
                    xts = [_load_x(2 * half)]
                    tblw = prc.tile([128, 2, 512], f32, tag="tbl", name="tblw")
                    for qloc in range(2):
                        nc.sync.dma_start(tblw[:, qloc, :], rope4[2 * half + qloc])
                    cosw = tblw[0:64]
                    r2a = tblw[64:96]
                    r2b = tblw[96:128]
                    # Each weight chunk loads once per half; its PSUM pairs the
                    # two quarters so RoPE runs at 1024 free elements per op.
                    for w3, dall, brow in (
                        (wq4, qt_all, bqrow),
                        (wk4, kt_all, bkrow),
                    ):
                        for mt in range(4):
                            wchA = paw.tile([128, KT // 2, 128], f32r,
                                            tag="wchA", name="wchA")
                            wchB = paw.tile([128, KT // 2, 128], f32r,
                                            tag="wchB", name="wchB")
                            nc.sync.dma_start(
                                wchA[:], w3[mt, :, 0:8, :].bitcast(f32r))
                            nc.sync.dma_start(
                                wchB[:], w3[mt, :, 8:16, :].bitcast(f32r))
                            if len(xts) == 1:
                                xts.append(_load_x(2 * half + 1))
                            ps = psa.tile([128, 2, 512], f32, tag="psa",
                                          name="ps_a")
                            for qloc in range(2):
                                xA, xB = xts[qloc]
                                for k in range(KT):
                                    wc = (wchA[:, k, :] if k < 8
                                          else wchB[:, k - 8, :])
                                    xv = (xA[:, k, :] if k < 8
                                          else xB[:, k - 8, :])
                                    nc.tensor.matmul(
                                        ps[:, qloc, :], wc, xv,
                                        start=(k == 0),
                                        stop=(k == KT - 1 and not WITH_BIAS),
                                    )
                                if WITH_BIAS:
                                    nc.tensor.matmul(
                                        ps[:, qloc, :],
                                        brow[:, mt * 128:(mt + 1) * 128],
                                        ones, start=False, stop=True,
                                    )
                            d3 = dall[:, mt, half * 1024:(half + 1) * 1024
                                      ].rearrange("p (a b) -> p a b", a=2)
                            nc.vector.tensor_mul(d3[0:64], ps[0:64], cosw)
                            nc.vector.tensor_mul(d3[64:128], ps[64:128], cosw)
                            # rotate-half * sin (sign folded into rsin)
                            tmp = prt.tile([128, 2, 512], f32r, tag="tmp",
                                           name="tmp")
                            for b0 in (0, 64):
                                nc.vector.tensor_mul(
                                    tmp[b0:b0 + 32], ps[b0 + 32:b0 + 64], r2a)
                                nc.vector.tensor_mul(
                                    tmp[b0 + 32:b0 + 64], ps[b0:b0 + 32], r2b)
                            nc.vector.tensor_add(d3[:], d3[:], tmp[:])

            with (
                tc.tile_pool(name="pav", bufs=1) as pav,
                tc.tile_pool(name="pa2", bufs=2) as pa2,
                tc.tile_pool(name="psv", bufs=3, space="PSUM") as psv,
            ):
                # va default 1.0 -> the per-head 65th column stays 1 (ones
                # column for the softmax denominator); data columns are
                # overwritten by the sigmoid copies below.
                for vg in range(4):
                    nc.sync.dma_start(
                        va_all[:, vg * 2080:(vg + 1) * 2080],
                        vinit[:, vg * 2080:(vg + 1) * 2080].bitcast(f32r))
                wvf = pav.tile([128, KT, CW], f32r, name="wvf")
                for kg in range(4):
                    nc.sync.dma_start(wvf[:, 4 * kg:4 * kg + 4, :],
                                      wv4[:, 4 * kg:4 * kg + 4, :].bitcast(f32r))
                for qtr in range(4):
                    xtqA = pa2.tile([128, KT // 2, 512], f32r, tag="xt2A",
                                    name="xtq2A")
                    xtqB = pa2.tile([128, KT // 2, 512], f32r, tag="xt2B",
                                    name="xtq2B")
                    for kg in range(2):
                        nc.sync.dma_start(
                            xtqA[:, 4 * kg:4 * kg + 4, :],
                            x4[qtr, :, 4 * kg:4 * kg + 4, :].bitcast(f32r))
                        nc.sync.dma_start(
                            xtqB[:, 4 * kg:4 * kg + 4, :],
                            x4[qtr, :, 8 + 4 * kg:8 + 4 * kg + 4, :].bitcast(f32r))
                    xk2 = lambda k: (xtqA[:, k, :] if k < 8 else xtqB[:, k - 8, :])
                    for st in range(4):
                        stg = qtr * 4 + st
                        ps = psv.tile([128, CW], f32, tag="psv", name="ps_v")
                        for k in range(KT):
                            nc.tensor.matmul(
                                ps[:], xk2(k)[:, st * 128:(st + 1) * 128],
                                wvf[:, k, :],
                                start=(k == 0),
                                stop=(k == KT - 1 and not WITH_BIAS),
                            )
                        if WITH_BIAS:
                            nc.tensor.matmul(
                                ps[:], ones[:, 0:128], bvt,
                                start=False, stop=True,
                            )
                        for h in range(H_PER_CORE):
                            nc.scalar.activation(
                                va[stg][:, 65 * h:65 * h + 64],
                                ps[:, 64 * h:64 * h + 64],
                                Act.Sigmoid,
                            )

            # ---------------- Phase B: causal attention ----------------
            with (
                tc.tile_pool(name="py", bufs=1) as py,
                tc.tile_pool(name="pc", bufs=2) as pc,
            ):
                ytr = [py.tile([128, S], f32r, name=f"ytr{i}") for i in range(4)]
                with (
                    tc.tile_pool(name="pb", bufs=1) as pb,
                    tc.tile_pool(name="pba", bufs=6) as pba,
                    tc.tile_pool(name="pbs", bufs=2) as pbs,
                    tc.tile_pool(name="pss", bufs=4, space="PSUM") as pss,
                    tc.tile_pool(name="psy", bufs=2, space="PSUM") as psy,
                    tc.tile_pool(name="pso", bufs=2, space="PSUM") as pso,
                ):
                    maskt = pb.tile([128, 4, 512], f32, name="maskt")
                    nc.sync.dma_start(maskt[:], masks[:])
                    kscr = [pb.tile([128, S], f32r, name=f"kscr{i}")
                            for i in range(2)]
                    # zero the pad halves (x0.0 of initialized data emits
                    # f32r-rounded zeros, which gpsimd memset cannot)
                    nc.vector.tensor_scalar_mul(kscr[0][64:128, :], qt[0][64:128, :], 0.0)
                    nc.vector.tensor_scalar_mul(kscr[1][0:64, :], qt[0][0:64, :], 0.0)

                    for qb in range(4):
                        nkt = 4 * qb + 4
                        for pi in range(4):
                            for hh in range(2):
                                lo, hi = hh * 64, (hh + 1) * 64
                                nc.vector.tensor_copy(
                                    kscr[hh][lo:hi, :], kt[pi][lo:hi, :]
                                )
                            for hh in range(2):
                                h = 2 * pi + hh
                                ks = kscr[hh]
                                lo, hi = hh * 64, (hh + 1) * 64
                                yps = psy.tile([65, 512], f32, tag="yps",
                                               name="ps_y")
                                for k_i in range(nkt):
                                    ps = pss.tile([128, 512], f32, tag="pss",
                                                  name="ps_s")
                                    nc.tensor.matmul(
                                        ps[:],
                                        ks[:, k_i * 128:(k_i + 1) * 128],
                                        qt[pi][:, qb * 512:(qb + 1) * 512],
                                        start=True, stop=True,
                                    )
                                    dt_i = k_i - 4 * qb
                                    if dt_i >= 0:
                                        nc.vector.tensor_add(
                                            ps[:], ps[:], maskt[:, dt_i, :]
                                        )
                                    at = pba.tile([128, 512], f32r, tag="at",
                                                  name="at")
                                    nc.scalar.activation(at[:], ps[:], Act.Exp)
                                    nc.tensor.matmul(
                                        yps[:],
                                        va[k_i][:, 65 * h:65 * h + 65],
                                        at[:],
                                        start=(k_i == 0), stop=(k_i == nkt - 1),
                                    )
                                den = pbs.tile([1, 512], f32, tag="den", name="den")
                                nc.vector.tensor_copy(den[:], yps[64:65, :])
                                rc = pbs.tile([1, 512], f32, tag="rc", name="rc")
                                nc.vector.reciprocal_approx_fast(rc[:], den[:])
                                s64 = pbs.tile([64, 512], f32, tag="s64",
                                               name="s64")
                                nc.gpsimd.partition_broadcast(s64[:], rc[:])
                                nc.vector.tensor_mul(
                                    ytr[pi][lo:hi, qb * 512:(qb + 1) * 512],
                                    yps[0:64, :], s64[:],
                                )
                        # o-projection rows for this query block are complete
                        for nt in range(4):
                            woc = pc.tile([128, 4, 512], f32r, tag="woc",
                                          name="woc")
                            for kc in range(4):
                                nc.sync.dma_start(
                                    woc[:, kc, :], wo4[nt, :, kc, :].bitcast(f32r))
                            for sl in range(4):
                                st = 4 * qb + sl
                                ps = pso.tile([128, 512], f32, tag="pso",
                                              name="ps_o")
                                for kc in range(4):
                                    nc.tensor.matmul(
                                        ps[:],
                                        ytr[kc][:, st * 128:(st + 1) * 128],
                                        woc[:, kc, :],
                                        start=(kc == 0), stop=(kc == 3),
                                    )
                                ostg = pc.tile([128, 512], f32, tag="ostg",
                                               name="ostg")
                                nc.scalar.copy(ostg[:], ps[:])
                                nc.sync.dma_start(
                                    part[st * 128:(st + 1) * 128,
                                         nt * 512:(nt + 1) * 512],
                                    ostg[:],
                                )

    nc.compile()
    return nc


def _rope_tables():
    half = DH // 2
    inv_freq = 1.0 / (10000.0 ** (np.arange(0, half, dtype=np.float32) / half))
    t = np.arange(S, dtype=np.float32)
    freqs = np.einsum("i,j->ij", t, inv_freq)            # [S, 32]
    emb = np.concatenate([freqs, freqs], axis=-1)        # [S, 64]
    cos = np.cos(emb).T.astype(np.float32)                        # [64, S]
    sin = np.sin(emb).T.astype(np.float32)
    rsin = np.concatenate([-sin[:32], sin[32:]], axis=0)
    return np.ascontiguousarray(np.concatenate([cos, rsin], axis=0))  # [128, S]


def _masks():
    j = np.arange(128)[:, None, None]
    dt = np.arange(4)[None, :, None]
    i = np.arange(512)[None, None, :]
    keep = (128 * dt + j) <= i
    return np.where(keep, 0.0, -1e30).astype(np.float32)  # [128, 4, 512]


def kernel(**inputs):
    global LAST_EXEC_NS
    x = np.asarray(inputs["x"], dtype=np.float32)
    Wq = np.asarray(inputs["Wq"], dtype=np.float32)
    Wk = np.asarray(inputs["Wk"], dtype=np.float32)
    Wv = np.asarray(inputs["Wv"], dtype=np.float32)
    Wo = np.asarray(inputs["Wo"], dtype=np.float32)
    bq = np.asarray(inputs["bq"], dtype=np.float32)
    bk = np.asarray(inputs["bk"], dtype=np.float32)
    bv = np.asarray(inputs["bv"], dtype=np.float32)
    bo = np.asarray(inputs["bo"], dtype=np.float32)

    ropeT = _rope_tables()
    masks = _masks()

    with_bias = any(float(np.abs(b).max()) > 0 for b in (bq, bk, bv))
    nc = _build(WITH_BIAS=with_bias)
    rope4 = np.ascontiguousarray(ropeT.reshape(128, 4, 512).transpose(1, 0, 2))
    vinit = np.ones((128, 16 * 520), dtype=np.float32)
    in_maps = []
    for c in range(N_CORES):
        b, g = c // 4, c % 4
        sl = slice(CW * g, CW * (g + 1))
        xT = x[b].T                                    # [D, S]
        x4 = np.ascontiguousarray(
            xT.reshape(KT, 128, 4, 512).transpose(2, 1, 0, 3))
        wq4 = np.ascontiguousarray(
            Wq[sl].T.reshape(KT, 128, 4, 128).transpose(2, 1, 0, 3))
        wk4 = np.ascontiguousarray(
            (Wk[sl].T * 0.125).reshape(KT, 128, 4, 128).transpose(2, 1, 0, 3))
        wv4 = np.ascontiguousarray(
            Wv[sl].T.reshape(KT, 128, CW).transpose(1, 0, 2))
        wo4 = np.ascontiguousarray(
            Wo[:, sl].T.reshape(4, 128, 4, 512).transpose(2, 1, 0, 3))
        in_maps.append({
            "x4": x4,
            "wq4": wq4,
            "wk4": wk4,
            "wv4": wv4,
            "wo4": wo4,
            "bq": np.ascontiguousarray(bq[sl].reshape(1, CW)),
            "bk": np.ascontiguousarray((bk[sl] * 0.125).reshape(1, CW)),
            "bv": np.ascontiguousarray(bv[sl].reshape(1, CW)),
            "rope4": rope4,
            "vinit": vinit,
            "masks": masks,
        })

    kwargs = {}
    if TRACE:
        kwargs = dict(trace=True, trace_cores=list(range(N_CORES)),
                      stitch_traces=False)
    global LAST_MEAN_NS
    r = run_bass_kernel_spmd(nc, in_maps, list(range(N_CORES)), **kwargs)
    LAST_EXEC_NS = r.exec_time_ns
    LAST_MEAN_NS = r.mean_exec_time_ns

    out = np.empty((B, S, D), dtype=np.float32)
    for b in range(B):
        acc = r.results[4 * b]["part"].astype(np.float32).copy()
        for g in range(1, 4):
            acc += r.results[4 * b + g]["part"]
        out[b] = acc + bo
    return out
